# revision 7
# baseline (speedup 1.0000x reference)
"""EnhancedMACDCell forward on 8 Trainium2 NeuronCores.

The reference computes, per batch row b of price_series [B, 64]:
    macd[b, j]  = w_fast . price[b, e-12:e] - w_slow . price[b, e-26:e]
                  + (b_fast - b_slow),        e = 64 - 8 + j, j = 0..8
    signal[b]   = w_sig . macd[b, :] + b_sig
    hist[b]     = macd[b, 8] - signal[b]
    out[b]      = tanh(hist[b] * norm_scale + norm_bias)

Everything before the tanh is linear in price_series, so the whole model
collapses to a single 64-tap linear functional per row:
    out[b] = tanh(price[b, :] . u + c0)
with u / c0 computed on the host (float64) from the tiny weight inputs.
Only columns 30..63 of u are nonzero, so the on-device compute is a
34-wide weighted row reduction + tanh over [1M, 64] float32 - purely
memory bound (32 MiB of HBM reads per core).

Kernel structure (v8, STT):
  - p-major sharding inside each core: partition p owns rows
    [p*T, (p+1)*T), so the output is per-partition contiguous and the
    final store is a handful of large descriptors.
  - all input chunk loads go on ONE HWDGE ring (sync) in chunk order:
    the ring is FIFO, so chunk completions arrive in order and a single
    semaphore serializes the pipeline with no inter-queue interleaving.
  - compute: the per-row 34-tap dot runs as 34 fused
    scalar_tensor_tensor ops (acc = x[:,:,c]*u[c] + acc) on DVE with
    the weights as compile-time immediates - roughly half the DVE
    element traffic of the tensor_mul + reduce_sum formulation, and no
    weight tile / weight DMA at all.
  - tanh on ACT per chunk; two batched output flushes on the scalar
    HWDGE ring.

Sharding: pure data parallel - 8 equal batch shards, weights replicated.
"""

import os
import sys

import numpy as np

for _p in ("/opt/trn_rl_repo", "/root/.axon_site/_ro/trn_rl_repo"):
    if os.path.isdir(_p) and _p not in sys.path:
        sys.path.insert(0, _p)

import concourse.bacc as bacc
import concourse.bass as bass
import concourse.mybir as mybir
from concourse import tile
from concourse.bass_utils import run_bass_kernel_spmd

FAST, SLOW, SIG = 12, 26, 9
S = 64
N_CORES = 8
P = 128           # SBUF partitions
C_LO, C_HI = 30, 64
C = C_HI - C_LO   # 34 columns with nonzero weight


def _collapsed_weights(w_fast, b_fast, w_slow, b_slow, w_sig, b_sig,
                       norm_scale, norm_bias):
    """Fold the whole linear pipeline into (u[64], c0)."""
    wf = np.asarray(w_fast, np.float64).reshape(-1)
    ws = np.asarray(w_slow, np.float64).reshape(-1)
    wg = np.asarray(w_sig, np.float64).reshape(-1)
    A = np.zeros((SIG, S), np.float64)
    for j in range(SIG):
        e = S - (SIG - 1) + j
        A[j, e - FAST:e] += wf
        A[j, e - SLOW:e] -= ws
    coeff = -wg.copy()
    coeff[SIG - 1] += 1.0
    u = coeff @ A
    c0 = (float(np.asarray(b_fast).reshape(-1)[0])
          - float(np.asarray(b_slow).reshape(-1)[0])) * coeff.sum() \
        - float(np.asarray(b_sig).reshape(-1)[0])
    ns = float(np.asarray(norm_scale).reshape(-1)[0])
    nb = float(np.asarray(norm_bias).reshape(-1)[0])
    return (u * ns).astype(np.float64), float(c0 * ns + nb)


def _build_scan(b_core: int, u, c0: float, sched=None, bufs: int = 3,
                flush_at: int | None = None) -> bass.Bass:
    """Full-row loads on one HWDGE ring; the per-row 34-tap dot computed
    by a single tensor_tensor_scan per chunk over the raw [P, ri*64]
    tile:  state = rho[c]*state + x[c]  with rho[c] = u[c-1]/u[c] inside
    the tap window, rho = 0 elsewhere (isolates rows, leaves junk in the
    dead columns).  state at column 63 is dot/u[63]; ACT finishes with
    tanh(state*u63 + c0) reading the strided column-63 view."""
    from contextlib import ExitStack

    T = b_core // P
    assert b_core % P == 0
    if sched is None:
        sched = [128] * (T // 128 - 1) + [64, 64]
    assert sum(sched) == T
    ri_max = max(sched)
    n_chunks = len(sched)
    offs = np.concatenate(([0], np.cumsum(sched))).astype(int)
    if flush_at is None:
        flush_at = n_chunks - 3
    flush_rows = int(offs[flush_at + 1])

    # rho pattern (64-periodic): rho[c] = u[c-1]/u[c] for C_LO < c < C_HI
    rho = [0.0] * S
    for c in range(C_LO + 1, C_HI):
        rho[c] = float(u[c - 1] / u[c])
    u_last = float(u[C_HI - 1])

    nc = bacc.Bacc()
    x = nc.declare_dram_parameter("x", [b_core, S], mybir.dt.float32,
                                  isOutput=False)
    y = nc.declare_dram_parameter("y", [b_core], mybir.dt.float32,
                                  isOutput=True)

    xv = x[:].rearrange("(p t) s -> p t s", p=P)   # [P, T, S]
    yv = y[:].rearrange("(p t) -> p t", p=P)       # [P, T]

    with ExitStack() as ctx:
        ef = ctx.enter_context
        xs = [ef(nc.sbuf_tensor(f"xs{k}", [P, ri_max * S], mybir.dt.float32))
              for k in range(bufs)]
        rp = ef(nc.sbuf_tensor("rp", [P, ri_max * S], mybir.dt.float32))
        rp1 = ef(nc.sbuf_tensor("rp1", [P, S], mybir.dt.float32))
        ot = ef(nc.sbuf_tensor("ot", [P, T], mybir.dt.float32))
        bt = ef(nc.sbuf_tensor("bt", [P, 1], mybir.dt.float32))
        st = ef(nc.sbuf_tensor("st", [P, 1], mybir.dt.float32))
        s_x = ef(nc.semaphore("s_x"))
        s_red = ef(nc.semaphore("s_red"))
        s_act = ef(nc.semaphore("s_act"))
        s_out = ef(nc.semaphore("s_out"))
        block = ef(nc.Block())

        def x3_view(i):
            ri = sched[i]
            return xs[i % bufs][:, :ri * S].rearrange("p (t s) -> p t s", s=S)

        @block.sync
        def _(sync):
            for i, ri in enumerate(sched):
                if i - bufs >= 0:
                    sync.wait_ge(s_act, i - bufs + 1)
                src = xv[:, offs[i]:offs[i] + ri, :]
                sync.dma_start(x3_view(i), src).then_inc(s_x, 16)

        @block.vector
        def _(vec):
            # build the rho pattern: one row, then broadcast to ri_max rows
            vec.memset(bt[:, :], c0)
            vec.memset(st[:, :], u_last)
            vec.memset(rp1[:, :], 0.0)
            for c in range(C_LO + 1, C_HI):
                if rho[c] != 0.0:
                    vec.memset(rp1[:, c:c + 1], rho[c])
            rp3 = rp[:, :].rearrange("p (t s) -> p t s", s=S)
            nc.vector.tensor_copy(
                rp3, rp1[:, :].unsqueeze(1).broadcast_to([P, ri_max, S]))
            mult = mybir.AluOpType.mult
            add = mybir.AluOpType.add
            for i, ri in enumerate(sched):
                vec.wait_ge(s_x, 16 * (i + 1))
                flat = xs[i % bufs][:, :ri * S]
                nc.vector.tensor_tensor_scan(
                    flat, rp[:, :ri * S], flat, 0.0, mult, add
                ).then_inc(s_red, 1)

        @block.scalar
        def _(act):
            for i, ri in enumerate(sched):
                act.wait_ge(s_red, i + 1)
                nc.scalar.activation(
                    ot[:, offs[i]:offs[i] + ri], x3_view(i)[:, :, S - 1],
                    mybir.ActivationFunctionType.Tanh,
                    bias=bt[:, 0:1], scale=st[:, 0:1]).then_inc(s_act, 1)
                if i == flush_at:
                    act.dma_start(yv[:, :flush_rows],
                                  ot[:, :flush_rows]).then_inc(s_out, 16)
            act.dma_start(yv[:, flush_rows:],
                          ot[:, flush_rows:]).then_inc(s_out, 16)
            act.wait_ge(s_out, 32)

    nc.compile()
    return nc


def _build_stt(b_core: int, u, c0: float, sched=None, bufs: int = 4,
               n_chains: int = 2, flush_at: int | None = None) -> bass.Bass:
    """STT pipeline: single input DMA ring, fused AXPY chains on DVE,
    per-chunk tanh on ACT, two batched output flushes."""
    from contextlib import ExitStack

    T = b_core // P
    assert b_core % P == 0
    if sched is None:
        assert T % 128 == 32 % 128 or T % 128 == 0
        sched = [32, 96] + [128] * ((T - 128) // 128)
    assert sum(sched) == T
    ri_max = max(sched)
    n_chunks = len(sched)
    offs = np.concatenate(([0], np.cumsum(sched))).astype(int)
    if flush_at is None:
        flush_at = n_chunks - 3
    flush_rows = int(offs[flush_at + 1])

    uc = [float(u[C_LO + c]) for c in range(C)]

    nc = bacc.Bacc()
    x = nc.declare_dram_parameter("x", [b_core, S], mybir.dt.float32,
                                  isOutput=False)
    y = nc.declare_dram_parameter("y", [b_core], mybir.dt.float32,
                                  isOutput=True)

    xv = x[:].rearrange("(p t) s -> p t s", p=P)   # [P, T, S]
    yv = y[:].rearrange("(p t) -> p t", p=P)       # [P, T]

    with ExitStack() as ctx:
        ef = ctx.enter_context
        xs = [ef(nc.sbuf_tensor(f"xs{k}", [P, ri_max * S], mybir.dt.float32))
              for k in range(bufs)]
        # two alternating accumulator sets, n_chains chains each
        accs = [[ef(nc.sbuf_tensor(f"acc{s}_{j}", [P, ri_max],
                                   mybir.dt.float32))
                 for j in range(n_chains)] for s in range(2)]
        ot = ef(nc.sbuf_tensor("ot", [P, T], mybir.dt.float32))
        s_x = ef(nc.semaphore("s_x"))
        s_red = ef(nc.semaphore("s_red"))
        s_act = ef(nc.semaphore("s_act"))
        s_out = ef(nc.semaphore("s_out"))
        block = ef(nc.Block())

        def x3_view(i):
            ri = sched[i]
            xt = xs[i % bufs]
            return xt[:, :ri * S].rearrange("p (t s) -> p t s", s=S)

        @block.sync
        def _(sync):
            for i, ri in enumerate(sched):
                if i - bufs >= 0:
                    sync.wait_ge(s_red, i - bufs + 1)
                src = xv[:, offs[i]:offs[i] + ri, :]
                sync.dma_start(x3_view(i), src).then_inc(s_x, 16)

        @block.vector
        def _(vec):
            add = mybir.AluOpType.add
            mult = mybir.AluOpType.mult
            for i, ri in enumerate(sched):
                vec.wait_ge(s_x, 16 * (i + 1))
                if i >= 2:
                    vec.wait_ge(s_act, i - 1)
                x3 = x3_view(i)
                acc = [a[:, :ri] for a in accs[i % 2]]
                for c in range(n_chains):
                    nc.vector.tensor_scalar(
                        acc[c], x3[:, :, C_LO + c], uc[c],
                        c0 if c == 0 else 0.0, mult, add)
                for c in range(n_chains, C):
                    j = c % n_chains
                    nc.vector.scalar_tensor_tensor(
                        acc[j], x3[:, :, C_LO + c], uc[c], acc[j],
                        mult, add)
                if n_chains == 1:
                    ins = nc.vector.tensor_copy(acc[0], acc[0])
                else:
                    ins = nc.vector.tensor_add(acc[0], acc[0], acc[1])
                    for j in range(2, n_chains):
                        ins = nc.vector.tensor_add(acc[0], acc[0], acc[j])
                ins.then_inc(s_red, 1)

        @block.scalar
        def _(act):
            for i, ri in enumerate(sched):
                act.wait_ge(s_red, i + 1)
                nc.scalar.activation(
                    ot[:, offs[i]:offs[i] + ri], accs[i % 2][0][:, :ri],
                    mybir.ActivationFunctionType.Tanh,
                    bias=0.0, scale=1.0).then_inc(s_act, 1)
                if i == flush_at:
                    act.dma_start(yv[:, :flush_rows],
                                  ot[:, :flush_rows]).then_inc(s_out, 16)
            act.dma_start(yv[:, flush_rows:],
                          ot[:, flush_rows:]).then_inc(s_out, 16)
            act.wait_ge(s_out, 32)

    nc.compile()
    return nc


def _build(b_core: int, c0: float, r: int = 64, bufs: int = 3,
           alt_queues: bool = True) -> bass.Bass:
    """Generic fallback (tile framework, mul+reduce) for odd shapes."""
    nc = bacc.Bacc()
    x = nc.declare_dram_parameter("x", [b_core, S], mybir.dt.float32,
                                  isOutput=False)
    w = nc.declare_dram_parameter("w", [P, C], mybir.dt.float32,
                                  isOutput=False)
    y = nc.declare_dram_parameter("y", [b_core], mybir.dt.float32,
                                  isOutput=True)

    rows_per_tile = P * r
    n_tiles = b_core // rows_per_tile
    assert b_core % rows_per_tile == 0

    xv = x[:].rearrange("(n p r) s -> n p r s", p=P, r=r)
    yv = y[:].rearrange("(n p r) -> n p r", p=P, r=r)

    with tile.TileContext(nc) as tc:
        with (
            tc.tile_pool(name="wp", bufs=1) as wp,
            tc.tile_pool(name="xp", bufs=bufs) as xp,
            tc.tile_pool(name="pp", bufs=2) as pp,
            tc.tile_pool(name="rp", bufs=2) as rp,
            tc.tile_pool(name="op", bufs=2) as op,
        ):
            wt = wp.tile([P, C], mybir.dt.float32)
            nc.sync.dma_start(wt[:], w[:])
            bt = wp.tile([P, 1], mybir.dt.float32, tag="bias")
            nc.vector.memset(bt[:], c0)
            for i in range(n_tiles):
                dma_eng = nc.scalar if (alt_queues and i % 2) else nc.sync
                xt = xp.tile([P, r * S], mybir.dt.float32)
                x3full = xt[:].rearrange("p (r s) -> p r s", s=S)
                dma_eng.dma_start(x3full, xv[i])
                x3 = x3full[:, :, C_LO:C_HI]
                pt = pp.tile([P, r * C], mybir.dt.float32)
                p3 = pt[:].rearrange("p (r c) -> p r c", c=C)
                wb = wt[:].unsqueeze(1).broadcast_to([P, r, C])
                nc.vector.tensor_mul(p3, x3, wb)
                rt = rp.tile([P, r], mybir.dt.float32)
                nc.vector.reduce_sum(rt[:], p3, axis=mybir.AxisListType.X)
                ot = op.tile([P, r], mybir.dt.float32)
                nc.scalar.activation(ot[:], rt[:],
                                     mybir.ActivationFunctionType.Tanh,
                                     bias=bt[:, 0:1], scale=1.0)
                nc.sync.dma_start(yv[i], ot[:])
    nc.compile()
    return nc


def _scan_safe(u) -> bool:
    """The ratio recurrence divides by every in-window u[c]; guard the
    amplification of intermediate states."""
    w = np.abs(np.asarray(u, np.float64)[C_LO:C_HI])
    if w.min() <= 0.0:
        return False
    amp = np.cumsum(w) / w
    return float(amp.max()) < 1e5


def build_main(b_core: int, u, c0: float):
    """The nc used for the standard 1M-row problem (also used by test.py
    for tracing)."""
    if _scan_safe(u):
        return _build_scan(b_core, u, c0)
    return _build_stt(b_core, u, c0)


def kernel(**inputs) -> np.ndarray:
    price = np.ascontiguousarray(np.asarray(inputs["price_series"],
                                            dtype=np.float32))
    B = price.shape[0]
    assert B % N_CORES == 0
    b_core = B // N_CORES

    u, c0 = _collapsed_weights(
        inputs["w_fast"], inputs["b_fast"], inputs["w_slow"],
        inputs["b_slow"], inputs["w_sig"], inputs["b_sig"],
        inputs["norm_scale"], inputs["norm_bias"])

    T = b_core // P if b_core % P == 0 else 0
    if T >= 256 and (T - 128) % 128 == 0:
        nc = build_main(b_core, u, c0)
        in_maps = [{"x": price[i * b_core:(i + 1) * b_core]}
                   for i in range(N_CORES)]
    else:
        nc = _build(b_core, c0, r=max(1, min(64, b_core // P)))
        w_rep = np.ascontiguousarray(
            np.broadcast_to(u[C_LO:C_HI][None, :].astype(np.float32), (P, C)))
        in_maps = [{"x": price[i * b_core:(i + 1) * b_core], "w": w_rep}
                   for i in range(N_CORES)]
    res = run_bass_kernel_spmd(nc, in_maps, list(range(N_CORES)))
    out = np.concatenate([res.results[i]["y"].reshape(-1)
                          for i in range(N_CORES)])
    return out.reshape(B, 1).astype(np.float32)


# revision 14
# speedup vs baseline: 1.4341x; 1.4341x over previous
"""EnhancedMACDCell forward on 8 Trainium2 NeuronCores.

The reference computes, per batch row b of price_series [B, 64]:
    macd[b, j]  = w_fast . price[b, e-12:e] - w_slow . price[b, e-26:e]
                  + (b_fast - b_slow),        e = 64 - 8 + j, j = 0..8
    signal[b]   = w_sig . macd[b, :] + b_sig
    hist[b]     = macd[b, 8] - signal[b]
    out[b]      = tanh(hist[b] * norm_scale + norm_bias)

Everything before the tanh is linear in price_series, so the whole model
collapses to a single 64-tap linear functional per row:
    out[b] = tanh(price[b, :] . u + c0)
with u / c0 computed on the host (float64) from the tiny weight inputs.
Only columns 30..63 of u are nonzero, so the on-device compute is a
34-wide weighted row reduction + tanh over [1M, 64] float32 - purely
memory bound (32 MiB of HBM reads per core).

Kernel structure (v8, STT):
  - p-major sharding inside each core: partition p owns rows
    [p*T, (p+1)*T), so the output is per-partition contiguous and the
    final store is a handful of large descriptors.
  - all input chunk loads go on ONE HWDGE ring (sync) in chunk order:
    the ring is FIFO, so chunk completions arrive in order and a single
    semaphore serializes the pipeline with no inter-queue interleaving.
  - compute: the per-row 34-tap dot runs as 34 fused
    scalar_tensor_tensor ops (acc = x[:,:,c]*u[c] + acc) on DVE with
    the weights as compile-time immediates - roughly half the DVE
    element traffic of the tensor_mul + reduce_sum formulation, and no
    weight tile / weight DMA at all.
  - tanh on ACT per chunk; two batched output flushes on the scalar
    HWDGE ring.

Sharding: pure data parallel - 8 equal batch shards, weights replicated.
"""

import os
import sys

import numpy as np

for _p in ("/opt/trn_rl_repo", "/root/.axon_site/_ro/trn_rl_repo"):
    if os.path.isdir(_p) and _p not in sys.path:
        sys.path.insert(0, _p)

import concourse.bacc as bacc
import concourse.bass as bass
import concourse.mybir as mybir
from concourse import tile
from concourse.bass_utils import run_bass_kernel_spmd

FAST, SLOW, SIG = 12, 26, 9
S = 64
N_CORES = 8
P = 128           # SBUF partitions
C_LO, C_HI = 30, 64
C = C_HI - C_LO   # 34 columns with nonzero weight


def _collapsed_weights(w_fast, b_fast, w_slow, b_slow, w_sig, b_sig,
                       norm_scale, norm_bias):
    """Fold the whole linear pipeline into (u[64], c0)."""
    wf = np.asarray(w_fast, np.float64).reshape(-1)
    ws = np.asarray(w_slow, np.float64).reshape(-1)
    wg = np.asarray(w_sig, np.float64).reshape(-1)
    A = np.zeros((SIG, S), np.float64)
    for j in range(SIG):
        e = S - (SIG - 1) + j
        A[j, e - FAST:e] += wf
        A[j, e - SLOW:e] -= ws
    coeff = -wg.copy()
    coeff[SIG - 1] += 1.0
    u = coeff @ A
    c0 = (float(np.asarray(b_fast).reshape(-1)[0])
          - float(np.asarray(b_slow).reshape(-1)[0])) * coeff.sum() \
        - float(np.asarray(b_sig).reshape(-1)[0])
    ns = float(np.asarray(norm_scale).reshape(-1)[0])
    nb = float(np.asarray(norm_bias).reshape(-1)[0])
    return (u * ns).astype(np.float64), float(c0 * ns + nb)


def _build_scan(b_core: int, u, c0: float, sched=None, bufs: int = 3,
                flush_at: int | None = None) -> bass.Bass:
    """Full-row loads on one HWDGE ring; the per-row 34-tap dot computed
    by a single tensor_tensor_scan per chunk over the raw [P, ri*64]
    tile:  state = rho[c]*state + x[c]  with rho[c] = u[c-1]/u[c] inside
    the tap window, rho = 0 elsewhere (isolates rows, leaves junk in the
    dead columns).  state at column 63 is dot/u[63]; ACT finishes with
    tanh(state*u63 + c0) reading the strided column-63 view."""
    from contextlib import ExitStack

    T = b_core // P
    assert b_core % P == 0
    if sched is None:
        sched = [128] * (T // 128 - 1) + [64, 64]
    assert sum(sched) == T
    ri_max = max(sched)
    n_chunks = len(sched)
    offs = np.concatenate(([0], np.cumsum(sched))).astype(int)
    if flush_at is None:
        flush_at = n_chunks - 3
    flush_rows = int(offs[flush_at + 1])

    # rho pattern (64-periodic): rho[c] = u[c-1]/u[c] for C_LO < c < C_HI
    rho = [0.0] * S
    for c in range(C_LO + 1, C_HI):
        rho[c] = float(u[c - 1] / u[c])
    u_last = float(u[C_HI - 1])

    nc = bacc.Bacc()
    x = nc.declare_dram_parameter("x", [b_core, S], mybir.dt.float32,
                                  isOutput=False)
    y = nc.declare_dram_parameter("y", [b_core], mybir.dt.float32,
                                  isOutput=True)

    xv = x[:].rearrange("(p t) s -> p t s", p=P)   # [P, T, S]
    yv = y[:].rearrange("(p t) -> p t", p=P)       # [P, T]

    with ExitStack() as ctx:
        ef = ctx.enter_context
        xs = [ef(nc.sbuf_tensor(f"xs{k}", [P, ri_max * S], mybir.dt.float32))
              for k in range(bufs)]
        rp = ef(nc.sbuf_tensor("rp", [P, ri_max * S], mybir.dt.float32))
        rp1 = ef(nc.sbuf_tensor("rp1", [P, S], mybir.dt.float32))
        ot = ef(nc.sbuf_tensor("ot", [P, T], mybir.dt.float32))
        bt = ef(nc.sbuf_tensor("bt", [P, 1], mybir.dt.float32))
        st = ef(nc.sbuf_tensor("st", [P, 1], mybir.dt.float32))
        s_x = ef(nc.semaphore("s_x"))
        s_red = ef(nc.semaphore("s_red"))
        s_act = ef(nc.semaphore("s_act"))
        s_out = ef(nc.semaphore("s_out"))
        block = ef(nc.Block())

        def x3_view(i):
            ri = sched[i]
            return xs[i % bufs][:, :ri * S].rearrange("p (t s) -> p t s", s=S)

        @block.sync
        def _(sync):
            for i, ri in enumerate(sched):
                if i - bufs >= 0:
                    sync.wait_ge(s_act, i - bufs + 1)
                src = xv[:, offs[i]:offs[i] + ri, :]
                sync.dma_start(x3_view(i), src).then_inc(s_x, 16)

        @block.vector
        def _(vec):
            # build the rho pattern: one row, then broadcast to ri_max rows
            vec.memset(bt[:, :], c0)
            vec.memset(st[:, :], u_last)
            vec.memset(rp1[:, :], 0.0)
            for c in range(C_LO + 1, C_HI):
                if rho[c] != 0.0:
                    vec.memset(rp1[:, c:c + 1], rho[c])
            rp3 = rp[:, :].rearrange("p (t s) -> p t s", s=S)
            nc.vector.tensor_copy(
                rp3, rp1[:, :].unsqueeze(1).broadcast_to([P, ri_max, S]))
            mult = mybir.AluOpType.mult
            add = mybir.AluOpType.add
            for i, ri in enumerate(sched):
                vec.wait_ge(s_x, 16 * (i + 1))
                flat = xs[i % bufs][:, :ri * S]
                nc.vector.tensor_tensor_scan(
                    flat, rp[:, :ri * S], flat, 0.0, mult, add
                ).then_inc(s_red, 1)

        @block.scalar
        def _(act):
            for i, ri in enumerate(sched):
                act.wait_ge(s_red, i + 1)
                nc.scalar.activation(
                    ot[:, offs[i]:offs[i] + ri], x3_view(i)[:, :, S - 1],
                    mybir.ActivationFunctionType.Tanh,
                    bias=bt[:, 0:1], scale=st[:, 0:1]).then_inc(s_act, 1)
                if i == flush_at:
                    act.dma_start(yv[:, :flush_rows],
                                  ot[:, :flush_rows]).then_inc(s_out, 16)
            act.dma_start(yv[:, flush_rows:],
                          ot[:, flush_rows:]).then_inc(s_out, 16)
            act.wait_ge(s_out, 32)

    nc.compile()
    return nc


def _build_split(b_core: int, u, c0: float, sched=None, bufs: int = 4,
                 gps_set=(2, 4, 6), flush_at: int | None = None) -> bass.Bass:
    """Full-row loads on one HWDGE ring (p-major). Per-chunk compute =
    tensor_mul + reduce_sum, split between DVE and GpSimd so neither is
    the critical path; reduce targets a [P, T] accumulator so there are
    no cross-chunk tile hazards. ACT applies tanh per chunk; two batched
    output flushes on the scalar ring."""
    from contextlib import ExitStack

    T = b_core // P
    assert b_core % P == 0
    if sched is None:
        sched = [128] * (T // 128 - 1) + [64, 32, 16, 16]
    assert sum(sched) == T
    ri_max = max(sched)
    n_chunks = len(sched)
    offs = np.concatenate(([0], np.cumsum(sched))).astype(int)
    if flush_at is None:
        flush_at = n_chunks - 4
    flush_rows = int(offs[flush_at + 1])
    gps_set = set(i for i in gps_set if i < n_chunks)
    eng_of = ["gps" if i in gps_set else "dve" for i in range(n_chunks)]
    # running count of gps-assigned chunks up to and including i
    ngps = np.cumsum([e == "gps" for e in eng_of]).astype(int)

    uc = [float(u[C_LO + c]) for c in range(C)]

    nc = bacc.Bacc()
    x = nc.declare_dram_parameter("x", [b_core, S], mybir.dt.float32,
                                  isOutput=False)
    y = nc.declare_dram_parameter("y", [b_core], mybir.dt.float32,
                                  isOutput=True)

    xv = x[:].rearrange("(p t) s -> p t s", p=P)   # [P, T, S]
    yv = y[:].rearrange("(p t) -> p t", p=P)       # [P, T]

    with ExitStack() as ctx:
        ef = ctx.enter_context
        xs = [ef(nc.sbuf_tensor(f"xs{k}", [P, ri_max * S], mybir.dt.float32))
              for k in range(bufs)]
        ptd = ef(nc.sbuf_tensor("ptd", [P, ri_max * C], mybir.dt.float32))
        ptg = ef(nc.sbuf_tensor("ptg", [P, ri_max * C], mybir.dt.float32))
        rt = ef(nc.sbuf_tensor("rt", [P, T], mybir.dt.float32))
        ot = ef(nc.sbuf_tensor("ot", [P, T], mybir.dt.float32))
        wt = ef(nc.sbuf_tensor("wt", [P, C], mybir.dt.float32))
        bt = ef(nc.sbuf_tensor("bt", [P, 1], mybir.dt.float32))
        s_x = ef(nc.semaphore("s_x"))
        s_w = ef(nc.semaphore("s_w"))
        s_red = ef(nc.semaphore("s_red"))
        s_mg = ef(nc.semaphore("s_mg"))
        s_act = ef(nc.semaphore("s_act"))
        s_out = ef(nc.semaphore("s_out"))
        block = ef(nc.Block())

        def x3_view(i):
            ri = sched[i]
            return xs[i % bufs][:, :ri * S].rearrange("p (t s) -> p t s", s=S)

        @block.sync
        def _(sync):
            for i, ri in enumerate(sched):
                if i - bufs >= 0:
                    sync.wait_ge(s_red, i - bufs + 1)
                src = xv[:, offs[i]:offs[i] + ri, :]
                sync.dma_start(x3_view(i), src).then_inc(s_x, 16)

        @block.vector
        def _(vec):
            vec.memset(bt[:, :], c0)
            for c in range(C - 1):
                vec.memset(wt[:, c:c + 1], uc[c])
            vec.memset(wt[:, C - 1:C], uc[C - 1]).then_inc(s_w, 1)
            # DVE: muls for its own chunks, reduces for every chunk
            for i, ri in enumerate(sched):
                p3view = (ptg if eng_of[i] == "gps" else ptd)[
                    :, :ri * C].rearrange("p (t c) -> p t c", c=C)
                if eng_of[i] == "gps":
                    vec.wait_ge(s_mg, int(ngps[i]))
                else:
                    vec.wait_ge(s_x, 16 * (i + 1))
                    x3 = x3_view(i)[:, :, C_LO:C_HI]
                    wb = wt[:, :].unsqueeze(1).broadcast_to([P, ri, C])
                    nc.vector.tensor_mul(p3view, x3, wb)
                nc.vector.tensor_reduce(
                    rt[:, offs[i]:offs[i] + ri], p3view,
                    axis=mybir.AxisListType.X,
                    op=mybir.AluOpType.add).then_inc(s_red, 1)

        @block.gpsimd
        def _(gps):
            prev_i = None
            for i, ri in enumerate(sched):
                if eng_of[i] != "gps":
                    continue
                gps.wait_ge(s_x, 16 * (i + 1))
                if prev_i is None:
                    gps.wait_ge(s_w, 1)
                else:
                    gps.wait_ge(s_red, prev_i + 1)
                x3 = x3_view(i)[:, :, C_LO:C_HI]
                p3 = ptg[:, :ri * C].rearrange("p (t c) -> p t c", c=C)
                wb = wt[:, :].unsqueeze(1).broadcast_to([P, ri, C])
                nc.gpsimd.tensor_mul(p3, x3, wb).then_inc(s_mg, 1)
                prev_i = i

        @block.scalar
        def _(act):
            for i, ri in enumerate(sched):
                act.wait_ge(s_red, i + 1)
                nc.scalar.activation(
                    ot[:, offs[i]:offs[i] + ri], rt[:, offs[i]:offs[i] + ri],
                    mybir.ActivationFunctionType.Tanh,
                    bias=bt[:, 0:1], scale=1.0).then_inc(s_act, 1)
                if i == flush_at:
                    act.dma_start(yv[:, :flush_rows],
                                  ot[:, :flush_rows]).then_inc(s_out, 16)
            act.dma_start(yv[:, flush_rows:],
                          ot[:, flush_rows:]).then_inc(s_out, 16)
            act.wait_ge(s_out, 32)

    nc.compile()
    return nc


def _build_stt(b_core: int, u, c0: float, sched=None, bufs: int = 4,
               n_chains: int = 2, flush_at: int | None = None) -> bass.Bass:
    """STT pipeline: single input DMA ring, fused AXPY chains on DVE,
    per-chunk tanh on ACT, two batched output flushes."""
    from contextlib import ExitStack

    T = b_core // P
    assert b_core % P == 0
    if sched is None:
        assert T % 128 == 32 % 128 or T % 128 == 0
        sched = [32, 96] + [128] * ((T - 128) // 128)
    assert sum(sched) == T
    ri_max = max(sched)
    n_chunks = len(sched)
    offs = np.concatenate(([0], np.cumsum(sched))).astype(int)
    if flush_at is None:
        flush_at = n_chunks - 3
    flush_rows = int(offs[flush_at + 1])

    uc = [float(u[C_LO + c]) for c in range(C)]

    nc = bacc.Bacc()
    x = nc.declare_dram_parameter("x", [b_core, S], mybir.dt.float32,
                                  isOutput=False)
    y = nc.declare_dram_parameter("y", [b_core], mybir.dt.float32,
                                  isOutput=True)

    xv = x[:].rearrange("(p t) s -> p t s", p=P)   # [P, T, S]
    yv = y[:].rearrange("(p t) -> p t", p=P)       # [P, T]

    with ExitStack() as ctx:
        ef = ctx.enter_context
        xs = [ef(nc.sbuf_tensor(f"xs{k}", [P, ri_max * S], mybir.dt.float32))
              for k in range(bufs)]
        # two alternating accumulator sets, n_chains chains each
        accs = [[ef(nc.sbuf_tensor(f"acc{s}_{j}", [P, ri_max],
                                   mybir.dt.float32))
                 for j in range(n_chains)] for s in range(2)]
        ot = ef(nc.sbuf_tensor("ot", [P, T], mybir.dt.float32))
        s_x = ef(nc.semaphore("s_x"))
        s_red = ef(nc.semaphore("s_red"))
        s_act = ef(nc.semaphore("s_act"))
        s_out = ef(nc.semaphore("s_out"))
        block = ef(nc.Block())

        def x3_view(i):
            ri = sched[i]
            xt = xs[i % bufs]
            return xt[:, :ri * S].rearrange("p (t s) -> p t s", s=S)

        @block.sync
        def _(sync):
            for i, ri in enumerate(sched):
                if i - bufs >= 0:
                    sync.wait_ge(s_red, i - bufs + 1)
                src = xv[:, offs[i]:offs[i] + ri, :]
                sync.dma_start(x3_view(i), src).then_inc(s_x, 16)

        @block.vector
        def _(vec):
            add = mybir.AluOpType.add
            mult = mybir.AluOpType.mult
            for i, ri in enumerate(sched):
                vec.wait_ge(s_x, 16 * (i + 1))
                if i >= 2:
                    vec.wait_ge(s_act, i - 1)
                x3 = x3_view(i)
                acc = [a[:, :ri] for a in accs[i % 2]]
                for c in range(n_chains):
                    nc.vector.tensor_scalar(
                        acc[c], x3[:, :, C_LO + c], uc[c],
                        c0 if c == 0 else 0.0, mult, add)
                for c in range(n_chains, C):
                    j = c % n_chains
                    nc.vector.scalar_tensor_tensor(
                        acc[j], x3[:, :, C_LO + c], uc[c], acc[j],
                        mult, add)
                if n_chains == 1:
                    ins = nc.vector.tensor_copy(acc[0], acc[0])
                else:
                    ins = nc.vector.tensor_add(acc[0], acc[0], acc[1])
                    for j in range(2, n_chains):
                        ins = nc.vector.tensor_add(acc[0], acc[0], acc[j])
                ins.then_inc(s_red, 1)

        @block.scalar
        def _(act):
            for i, ri in enumerate(sched):
                act.wait_ge(s_red, i + 1)
                nc.scalar.activation(
                    ot[:, offs[i]:offs[i] + ri], accs[i % 2][0][:, :ri],
                    mybir.ActivationFunctionType.Tanh,
                    bias=0.0, scale=1.0).then_inc(s_act, 1)
                if i == flush_at:
                    act.dma_start(yv[:, :flush_rows],
                                  ot[:, :flush_rows]).then_inc(s_out, 16)
            act.dma_start(yv[:, flush_rows:],
                          ot[:, flush_rows:]).then_inc(s_out, 16)
            act.wait_ge(s_out, 32)

    nc.compile()
    return nc


def _build(b_core: int, c0: float, r: int = 64, bufs: int = 3,
           alt_queues: bool = True) -> bass.Bass:
    """Generic fallback (tile framework, mul+reduce) for odd shapes."""
    nc = bacc.Bacc()
    x = nc.declare_dram_parameter("x", [b_core, S], mybir.dt.float32,
                                  isOutput=False)
    w = nc.declare_dram_parameter("w", [P, C], mybir.dt.float32,
                                  isOutput=False)
    y = nc.declare_dram_parameter("y", [b_core], mybir.dt.float32,
                                  isOutput=True)

    rows_per_tile = P * r
    n_tiles = b_core // rows_per_tile
    assert b_core % rows_per_tile == 0

    xv = x[:].rearrange("(n p r) s -> n p r s", p=P, r=r)
    yv = y[:].rearrange("(n p r) -> n p r", p=P, r=r)

    with tile.TileContext(nc) as tc:
        with (
            tc.tile_pool(name="wp", bufs=1) as wp,
            tc.tile_pool(name="xp", bufs=bufs) as xp,
            tc.tile_pool(name="pp", bufs=2) as pp,
            tc.tile_pool(name="rp", bufs=2) as rp,
            tc.tile_pool(name="op", bufs=2) as op,
        ):
            wt = wp.tile([P, C], mybir.dt.float32)
            nc.sync.dma_start(wt[:], w[:])
            bt = wp.tile([P, 1], mybir.dt.float32, tag="bias")
            nc.vector.memset(bt[:], c0)
            for i in range(n_tiles):
                dma_eng = nc.scalar if (alt_queues and i % 2) else nc.sync
                xt = xp.tile([P, r * S], mybir.dt.float32)
                x3full = xt[:].rearrange("p (r s) -> p r s", s=S)
                dma_eng.dma_start(x3full, xv[i])
                x3 = x3full[:, :, C_LO:C_HI]
                pt = pp.tile([P, r * C], mybir.dt.float32)
                p3 = pt[:].rearrange("p (r c) -> p r c", c=C)
                wb = wt[:].unsqueeze(1).broadcast_to([P, r, C])
                nc.vector.tensor_mul(p3, x3, wb)
                rt = rp.tile([P, r], mybir.dt.float32)
                nc.vector.reduce_sum(rt[:], p3, axis=mybir.AxisListType.X)
                ot = op.tile([P, r], mybir.dt.float32)
                nc.scalar.activation(ot[:], rt[:],
                                     mybir.ActivationFunctionType.Tanh,
                                     bias=bt[:, 0:1], scale=1.0)
                nc.sync.dma_start(yv[i], ot[:])
    nc.compile()
    return nc


def _scan_safe(u) -> bool:
    """The ratio recurrence divides by every in-window u[c]; guard the
    amplification of intermediate states."""
    w = np.abs(np.asarray(u, np.float64)[C_LO:C_HI])
    if w.min() <= 0.0:
        return False
    amp = np.cumsum(w) / w
    return float(amp.max()) < 1e5


def build_main(b_core: int, u, c0: float):
    """The nc used for the standard 1M-row problem (also used by test.py
    for tracing)."""
    return _build_split(b_core, u, c0)


def kernel(**inputs) -> np.ndarray:
    price = np.ascontiguousarray(np.asarray(inputs["price_series"],
                                            dtype=np.float32))
    B = price.shape[0]
    assert B % N_CORES == 0
    b_core = B // N_CORES

    u, c0 = _collapsed_weights(
        inputs["w_fast"], inputs["b_fast"], inputs["w_slow"],
        inputs["b_slow"], inputs["w_sig"], inputs["b_sig"],
        inputs["norm_scale"], inputs["norm_bias"])

    T = b_core // P if b_core % P == 0 else 0
    if T >= 256 and (T - 128) % 128 == 0:
        nc = build_main(b_core, u, c0)
        in_maps = [{"x": price[i * b_core:(i + 1) * b_core]}
                   for i in range(N_CORES)]
    else:
        nc = _build(b_core, c0, r=max(1, min(64, b_core // P)))
        w_rep = np.ascontiguousarray(
            np.broadcast_to(u[C_LO:C_HI][None, :].astype(np.float32), (P, C)))
        in_maps = [{"x": price[i * b_core:(i + 1) * b_core], "w": w_rep}
                   for i in range(N_CORES)]
    res = run_bass_kernel_spmd(nc, in_maps, list(range(N_CORES)))
    out = np.concatenate([res.results[i]["y"].reshape(-1)
                          for i in range(N_CORES)])
    return out.reshape(B, 1).astype(np.float32)


# revision 18
# speedup vs baseline: 1.6300x; 1.1366x over previous
"""EnhancedMACDCell forward on 8 Trainium2 NeuronCores.

The reference computes, per batch row b of price_series [B, 64]:
    macd[b, j]  = w_fast . price[b, e-12:e] - w_slow . price[b, e-26:e]
                  + (b_fast - b_slow),        e = 64 - 8 + j, j = 0..8
    signal[b]   = w_sig . macd[b, :] + b_sig
    hist[b]     = macd[b, 8] - signal[b]
    out[b]      = tanh(hist[b] * norm_scale + norm_bias)

Everything before the tanh is linear in price_series, so the whole model
collapses to a single 64-tap linear functional per row:
    out[b] = tanh(price[b, :] . u + c0)
with u / c0 computed on the host (float64) from the tiny weight inputs.
Only columns 30..63 of u are nonzero, so the on-device compute is a
34-wide weighted row reduction + tanh over [1M, 64] float32 - purely
memory bound (32 MiB of HBM reads per core).

Kernel structure (v8, STT):
  - p-major sharding inside each core: partition p owns rows
    [p*T, (p+1)*T), so the output is per-partition contiguous and the
    final store is a handful of large descriptors.
  - all input chunk loads go on ONE HWDGE ring (sync) in chunk order:
    the ring is FIFO, so chunk completions arrive in order and a single
    semaphore serializes the pipeline with no inter-queue interleaving.
  - compute: the per-row 34-tap dot runs as 34 fused
    scalar_tensor_tensor ops (acc = x[:,:,c]*u[c] + acc) on DVE with
    the weights as compile-time immediates - roughly half the DVE
    element traffic of the tensor_mul + reduce_sum formulation, and no
    weight tile / weight DMA at all.
  - tanh on ACT per chunk; two batched output flushes on the scalar
    HWDGE ring.

Sharding: pure data parallel - 8 equal batch shards, weights replicated.
"""

import os
import sys

import numpy as np

for _p in ("/opt/trn_rl_repo", "/root/.axon_site/_ro/trn_rl_repo"):
    if os.path.isdir(_p) and _p not in sys.path:
        sys.path.insert(0, _p)

import concourse.bacc as bacc
import concourse.bass as bass
import concourse.mybir as mybir
from concourse import tile
from concourse.bass_utils import run_bass_kernel_spmd

FAST, SLOW, SIG = 12, 26, 9
S = 64
N_CORES = 8
P = 128           # SBUF partitions
C_LO, C_HI = 30, 64
C = C_HI - C_LO   # 34 columns with nonzero weight


def _collapsed_weights(w_fast, b_fast, w_slow, b_slow, w_sig, b_sig,
                       norm_scale, norm_bias):
    """Fold the whole linear pipeline into (u[64], c0)."""
    wf = np.asarray(w_fast, np.float64).reshape(-1)
    ws = np.asarray(w_slow, np.float64).reshape(-1)
    wg = np.asarray(w_sig, np.float64).reshape(-1)
    A = np.zeros((SIG, S), np.float64)
    for j in range(SIG):
        e = S - (SIG - 1) + j
        A[j, e - FAST:e] += wf
        A[j, e - SLOW:e] -= ws
    coeff = -wg.copy()
    coeff[SIG - 1] += 1.0
    u = coeff @ A
    c0 = (float(np.asarray(b_fast).reshape(-1)[0])
          - float(np.asarray(b_slow).reshape(-1)[0])) * coeff.sum() \
        - float(np.asarray(b_sig).reshape(-1)[0])
    ns = float(np.asarray(norm_scale).reshape(-1)[0])
    nb = float(np.asarray(norm_bias).reshape(-1)[0])
    return (u * ns).astype(np.float64), float(c0 * ns + nb)


def _build_scan(b_core: int, u, c0: float, sched=None, bufs: int = 3,
                flush_at: int | None = None) -> bass.Bass:
    """Full-row loads on one HWDGE ring; the per-row 34-tap dot computed
    by a single tensor_tensor_scan per chunk over the raw [P, ri*64]
    tile:  state = rho[c]*state + x[c]  with rho[c] = u[c-1]/u[c] inside
    the tap window, rho = 0 elsewhere (isolates rows, leaves junk in the
    dead columns).  state at column 63 is dot/u[63]; ACT finishes with
    tanh(state*u63 + c0) reading the strided column-63 view."""
    from contextlib import ExitStack

    T = b_core // P
    assert b_core % P == 0
    if sched is None:
        sched = [128] * (T // 128 - 1) + [64, 64]
    assert sum(sched) == T
    ri_max = max(sched)
    n_chunks = len(sched)
    offs = np.concatenate(([0], np.cumsum(sched))).astype(int)
    if flush_at is None:
        flush_at = n_chunks - 3
    flush_rows = int(offs[flush_at + 1])

    # rho pattern (64-periodic): rho[c] = u[c-1]/u[c] for C_LO < c < C_HI
    rho = [0.0] * S
    for c in range(C_LO + 1, C_HI):
        rho[c] = float(u[c - 1] / u[c])
    u_last = float(u[C_HI - 1])

    nc = bacc.Bacc()
    x = nc.declare_dram_parameter("x", [b_core, S], mybir.dt.float32,
                                  isOutput=False)
    y = nc.declare_dram_parameter("y", [b_core], mybir.dt.float32,
                                  isOutput=True)

    xv = x[:].rearrange("(p t) s -> p t s", p=P)   # [P, T, S]
    yv = y[:].rearrange("(p t) -> p t", p=P)       # [P, T]

    with ExitStack() as ctx:
        ef = ctx.enter_context
        xs = [ef(nc.sbuf_tensor(f"xs{k}", [P, ri_max * S], mybir.dt.float32))
              for k in range(bufs)]
        rp = ef(nc.sbuf_tensor("rp", [P, ri_max * S], mybir.dt.float32))
        rp1 = ef(nc.sbuf_tensor("rp1", [P, S], mybir.dt.float32))
        ot = ef(nc.sbuf_tensor("ot", [P, T], mybir.dt.float32))
        bt = ef(nc.sbuf_tensor("bt", [P, 1], mybir.dt.float32))
        st = ef(nc.sbuf_tensor("st", [P, 1], mybir.dt.float32))
        s_x = ef(nc.semaphore("s_x"))
        s_red = ef(nc.semaphore("s_red"))
        s_act = ef(nc.semaphore("s_act"))
        s_out = ef(nc.semaphore("s_out"))
        block = ef(nc.Block())

        def x3_view(i):
            ri = sched[i]
            return xs[i % bufs][:, :ri * S].rearrange("p (t s) -> p t s", s=S)

        @block.sync
        def _(sync):
            for i, ri in enumerate(sched):
                if i - bufs >= 0:
                    sync.wait_ge(s_act, i - bufs + 1)
                src = xv[:, offs[i]:offs[i] + ri, :]
                sync.dma_start(x3_view(i), src).then_inc(s_x, 16)

        @block.vector
        def _(vec):
            # build the rho pattern: one row, then broadcast to ri_max rows
            vec.memset(bt[:, :], c0)
            vec.memset(st[:, :], u_last)
            vec.memset(rp1[:, :], 0.0)
            for c in range(C_LO + 1, C_HI):
                if rho[c] != 0.0:
                    vec.memset(rp1[:, c:c + 1], rho[c])
            rp3 = rp[:, :].rearrange("p (t s) -> p t s", s=S)
            nc.vector.tensor_copy(
                rp3, rp1[:, :].unsqueeze(1).broadcast_to([P, ri_max, S]))
            mult = mybir.AluOpType.mult
            add = mybir.AluOpType.add
            for i, ri in enumerate(sched):
                vec.wait_ge(s_x, 16 * (i + 1))
                flat = xs[i % bufs][:, :ri * S]
                nc.vector.tensor_tensor_scan(
                    flat, rp[:, :ri * S], flat, 0.0, mult, add
                ).then_inc(s_red, 1)

        @block.scalar
        def _(act):
            for i, ri in enumerate(sched):
                act.wait_ge(s_red, i + 1)
                nc.scalar.activation(
                    ot[:, offs[i]:offs[i] + ri], x3_view(i)[:, :, S - 1],
                    mybir.ActivationFunctionType.Tanh,
                    bias=bt[:, 0:1], scale=st[:, 0:1]).then_inc(s_act, 1)
                if i == flush_at:
                    act.dma_start(yv[:, :flush_rows],
                                  ot[:, :flush_rows]).then_inc(s_out, 16)
            act.dma_start(yv[:, flush_rows:],
                          ot[:, flush_rows:]).then_inc(s_out, 16)
            act.wait_ge(s_out, 32)

    nc.compile()
    return nc


def _build_split(b_core: int, u, c0: float, sched=None, bufs: int = 4,
                 gps_set=(2, 4, 6), flush_at: int | None = None) -> bass.Bass:
    """Full-row loads on one HWDGE ring (p-major). Per-chunk compute =
    tensor_mul + reduce_sum, split between DVE and GpSimd so neither is
    the critical path; reduce targets a [P, T] accumulator so there are
    no cross-chunk tile hazards. ACT applies tanh per chunk; two batched
    output flushes on the scalar ring."""
    from contextlib import ExitStack

    T = b_core // P
    assert b_core % P == 0
    if sched is None:
        sched = [32, 64] + [96] * ((T - 256) // 96) + [64, 48, 32, 16]
    assert sum(sched) == T
    ri_max = max(sched)
    n_chunks = len(sched)
    offs = np.concatenate(([0], np.cumsum(sched))).astype(int)
    if flush_at is None:
        flush_at = n_chunks - 4
    flush_rows = int(offs[flush_at + 1])
    gps_set = set(i for i in gps_set if i < n_chunks)
    eng_of = ["gps" if i in gps_set else "dve" for i in range(n_chunks)]
    # per-engine running counts of completed chunks up to and incl. i
    ndve = np.cumsum([e == "dve" for e in eng_of]).astype(int)
    ngps = np.cumsum([e == "gps" for e in eng_of]).astype(int)

    uc = [float(u[C_LO + c]) for c in range(C)]

    nc = bacc.Bacc()
    x = nc.declare_dram_parameter("x", [b_core, S], mybir.dt.float32,
                                  isOutput=False)
    y = nc.declare_dram_parameter("y", [b_core], mybir.dt.float32,
                                  isOutput=True)

    xv = x[:].rearrange("(p t) s -> p t s", p=P)   # [P, T, S]
    yv = y[:].rearrange("(p t) -> p t", p=P)       # [P, T]

    with ExitStack() as ctx:
        ef = ctx.enter_context
        xs = [ef(nc.sbuf_tensor(f"xs{k}", [P, ri_max * S], mybir.dt.float32))
              for k in range(bufs)]
        ptd = ef(nc.sbuf_tensor("ptd", [P, ri_max * C], mybir.dt.float32))
        ptg = ef(nc.sbuf_tensor("ptg", [P, ri_max * C], mybir.dt.float32))
        rt = ef(nc.sbuf_tensor("rt", [P, T], mybir.dt.float32))
        ot = ef(nc.sbuf_tensor("ot", [P, T], mybir.dt.float32))
        wt = ef(nc.sbuf_tensor("wt", [P, C], mybir.dt.float32))
        bt = ef(nc.sbuf_tensor("bt", [P, 1], mybir.dt.float32))
        s_x = ef(nc.semaphore("s_x"))
        s_w = ef(nc.semaphore("s_w"))
        s_red = ef(nc.semaphore("s_red"))
        s_gc = ef(nc.semaphore("s_gc"))
        s_act = ef(nc.semaphore("s_act"))
        s_out = ef(nc.semaphore("s_out"))
        block = ef(nc.Block())

        def x3_view(i):
            ri = sched[i]
            return xs[i % bufs][:, :ri * S].rearrange("p (t s) -> p t s", s=S)

        def wait_done(engine, upto):
            """Wait until every chunk <= upto has finished compute."""
            if upto < 0:
                return
            if ndve[upto] > 0:
                engine.wait_ge(s_red, int(ndve[upto]))
            if ngps[upto] > 0:
                engine.wait_ge(s_gc, int(ngps[upto]))

        @block.sync
        def _(sync):
            for i, ri in enumerate(sched):
                if i - bufs >= 0:
                    wait_done(sync, i - bufs)
                src = xv[:, offs[i]:offs[i] + ri, :]
                sync.dma_start(x3_view(i), src).then_inc(s_x, 16)

        @block.vector
        def _(vec):
            vec.memset(bt[:, :], c0)
            for c in range(C - 1):
                vec.memset(wt[:, c:c + 1], uc[c])
            vec.memset(wt[:, C - 1:C], uc[C - 1]).then_inc(s_w, 2)
            for i, ri in enumerate(sched):
                if eng_of[i] != "dve":
                    continue
                vec.wait_ge(s_x, 16 * (i + 1))
                x3 = x3_view(i)[:, :, C_LO:C_HI]
                p3 = ptd[:, :ri * C].rearrange("p (t c) -> p t c", c=C)
                wb = wt[:, :].unsqueeze(1).broadcast_to([P, ri, C])
                nc.vector.tensor_mul(p3, x3, wb)
                nc.vector.tensor_reduce(
                    rt[:, offs[i]:offs[i] + ri], p3,
                    axis=mybir.AxisListType.X,
                    op=mybir.AluOpType.add).then_inc(s_red, 1)

        @block.gpsimd
        def _(gps):
            # fully independent: mul + binary add-tree, writes rt directly
            first = True
            for i, ri in enumerate(sched):
                if eng_of[i] != "gps":
                    continue
                gps.wait_ge(s_x, 16 * (i + 1))
                if first:
                    gps.wait_ge(s_w, 2)
                    first = False
                x3 = x3_view(i)[:, :, C_LO:C_HI]
                p3 = ptg[:, :ri * C].rearrange("p (t c) -> p t c", c=C)
                wb = wt[:, :].unsqueeze(1).broadcast_to([P, ri, C])
                nc.gpsimd.tensor_mul(p3, x3, wb)
                nc.gpsimd.tensor_add(p3[:, :, 0:17], p3[:, :, 0:17],
                                     p3[:, :, 17:34])
                nc.gpsimd.tensor_add(p3[:, :, 0:8], p3[:, :, 0:8],
                                     p3[:, :, 8:16])
                nc.gpsimd.tensor_add(p3[:, :, 0:4], p3[:, :, 0:4],
                                     p3[:, :, 4:8])
                nc.gpsimd.tensor_add(p3[:, :, 0:2], p3[:, :, 0:2],
                                     p3[:, :, 2:4])
                nc.gpsimd.tensor_add(p3[:, :, 0:1], p3[:, :, 0:1],
                                     p3[:, :, 1:2])
                nc.gpsimd.tensor_add(rt[:, offs[i]:offs[i] + ri],
                                     p3[:, :, 0], p3[:, :, 16]
                                     ).then_inc(s_gc, 1)

        @block.scalar
        def _(act):
            for i, ri in enumerate(sched):
                wait_done(act, i)
                nc.scalar.activation(
                    ot[:, offs[i]:offs[i] + ri], rt[:, offs[i]:offs[i] + ri],
                    mybir.ActivationFunctionType.Tanh,
                    bias=bt[:, 0:1], scale=1.0).then_inc(s_act, 1)
                if i == flush_at:
                    act.dma_start(yv[:, :flush_rows],
                                  ot[:, :flush_rows]).then_inc(s_out, 16)
            act.dma_start(yv[:, flush_rows:],
                          ot[:, flush_rows:]).then_inc(s_out, 16)
            act.wait_ge(s_out, 32)

    nc.compile()
    return nc


def _build_stt(b_core: int, u, c0: float, sched=None, bufs: int = 4,
               n_chains: int = 2, flush_at: int | None = None) -> bass.Bass:
    """STT pipeline: single input DMA ring, fused AXPY chains on DVE,
    per-chunk tanh on ACT, two batched output flushes."""
    from contextlib import ExitStack

    T = b_core // P
    assert b_core % P == 0
    if sched is None:
        assert T % 128 == 32 % 128 or T % 128 == 0
        sched = [32, 96] + [128] * ((T - 128) // 128)
    assert sum(sched) == T
    ri_max = max(sched)
    n_chunks = len(sched)
    offs = np.concatenate(([0], np.cumsum(sched))).astype(int)
    if flush_at is None:
        flush_at = n_chunks - 3
    flush_rows = int(offs[flush_at + 1])

    uc = [float(u[C_LO + c]) for c in range(C)]

    nc = bacc.Bacc()
    x = nc.declare_dram_parameter("x", [b_core, S], mybir.dt.float32,
                                  isOutput=False)
    y = nc.declare_dram_parameter("y", [b_core], mybir.dt.float32,
                                  isOutput=True)

    xv = x[:].rearrange("(p t) s -> p t s", p=P)   # [P, T, S]
    yv = y[:].rearrange("(p t) -> p t", p=P)       # [P, T]

    with ExitStack() as ctx:
        ef = ctx.enter_context
        xs = [ef(nc.sbuf_tensor(f"xs{k}", [P, ri_max * S], mybir.dt.float32))
              for k in range(bufs)]
        # two alternating accumulator sets, n_chains chains each
        accs = [[ef(nc.sbuf_tensor(f"acc{s}_{j}", [P, ri_max],
                                   mybir.dt.float32))
                 for j in range(n_chains)] for s in range(2)]
        ot = ef(nc.sbuf_tensor("ot", [P, T], mybir.dt.float32))
        s_x = ef(nc.semaphore("s_x"))
        s_red = ef(nc.semaphore("s_red"))
        s_act = ef(nc.semaphore("s_act"))
        s_out = ef(nc.semaphore("s_out"))
        block = ef(nc.Block())

        def x3_view(i):
            ri = sched[i]
            xt = xs[i % bufs]
            return xt[:, :ri * S].rearrange("p (t s) -> p t s", s=S)

        @block.sync
        def _(sync):
            for i, ri in enumerate(sched):
                if i - bufs >= 0:
                    sync.wait_ge(s_red, i - bufs + 1)
                src = xv[:, offs[i]:offs[i] + ri, :]
                sync.dma_start(x3_view(i), src).then_inc(s_x, 16)

        @block.vector
        def _(vec):
            add = mybir.AluOpType.add
            mult = mybir.AluOpType.mult
            for i, ri in enumerate(sched):
                vec.wait_ge(s_x, 16 * (i + 1))
                if i >= 2:
                    vec.wait_ge(s_act, i - 1)
                x3 = x3_view(i)
                acc = [a[:, :ri] for a in accs[i % 2]]
                for c in range(n_chains):
                    nc.vector.tensor_scalar(
                        acc[c], x3[:, :, C_LO + c], uc[c],
                        c0 if c == 0 else 0.0, mult, add)
                for c in range(n_chains, C):
                    j = c % n_chains
                    nc.vector.scalar_tensor_tensor(
                        acc[j], x3[:, :, C_LO + c], uc[c], acc[j],
                        mult, add)
                if n_chains == 1:
                    ins = nc.vector.tensor_copy(acc[0], acc[0])
                else:
                    ins = nc.vector.tensor_add(acc[0], acc[0], acc[1])
                    for j in range(2, n_chains):
                        ins = nc.vector.tensor_add(acc[0], acc[0], acc[j])
                ins.then_inc(s_red, 1)

        @block.scalar
        def _(act):
            for i, ri in enumerate(sched):
                act.wait_ge(s_red, i + 1)
                nc.scalar.activation(
                    ot[:, offs[i]:offs[i] + ri], accs[i % 2][0][:, :ri],
                    mybir.ActivationFunctionType.Tanh,
                    bias=0.0, scale=1.0).then_inc(s_act, 1)
                if i == flush_at:
                    act.dma_start(yv[:, :flush_rows],
                                  ot[:, :flush_rows]).then_inc(s_out, 16)
            act.dma_start(yv[:, flush_rows:],
                          ot[:, flush_rows:]).then_inc(s_out, 16)
            act.wait_ge(s_out, 32)

    nc.compile()
    return nc


def _build(b_core: int, c0: float, r: int = 64, bufs: int = 3,
           alt_queues: bool = True) -> bass.Bass:
    """Generic fallback (tile framework, mul+reduce) for odd shapes."""
    nc = bacc.Bacc()
    x = nc.declare_dram_parameter("x", [b_core, S], mybir.dt.float32,
                                  isOutput=False)
    w = nc.declare_dram_parameter("w", [P, C], mybir.dt.float32,
                                  isOutput=False)
    y = nc.declare_dram_parameter("y", [b_core], mybir.dt.float32,
                                  isOutput=True)

    rows_per_tile = P * r
    n_tiles = b_core // rows_per_tile
    assert b_core % rows_per_tile == 0

    xv = x[:].rearrange("(n p r) s -> n p r s", p=P, r=r)
    yv = y[:].rearrange("(n p r) -> n p r", p=P, r=r)

    with tile.TileContext(nc) as tc:
        with (
            tc.tile_pool(name="wp", bufs=1) as wp,
            tc.tile_pool(name="xp", bufs=bufs) as xp,
            tc.tile_pool(name="pp", bufs=2) as pp,
            tc.tile_pool(name="rp", bufs=2) as rp,
            tc.tile_pool(name="op", bufs=2) as op,
        ):
            wt = wp.tile([P, C], mybir.dt.float32)
            nc.sync.dma_start(wt[:], w[:])
            bt = wp.tile([P, 1], mybir.dt.float32, tag="bias")
            nc.vector.memset(bt[:], c0)
            for i in range(n_tiles):
                dma_eng = nc.scalar if (alt_queues and i % 2) else nc.sync
                xt = xp.tile([P, r * S], mybir.dt.float32)
                x3full = xt[:].rearrange("p (r s) -> p r s", s=S)
                dma_eng.dma_start(x3full, xv[i])
                x3 = x3full[:, :, C_LO:C_HI]
                pt = pp.tile([P, r * C], mybir.dt.float32)
                p3 = pt[:].rearrange("p (r c) -> p r c", c=C)
                wb = wt[:].unsqueeze(1).broadcast_to([P, r, C])
                nc.vector.tensor_mul(p3, x3, wb)
                rt = rp.tile([P, r], mybir.dt.float32)
                nc.vector.reduce_sum(rt[:], p3, axis=mybir.AxisListType.X)
                ot = op.tile([P, r], mybir.dt.float32)
                nc.scalar.activation(ot[:], rt[:],
                                     mybir.ActivationFunctionType.Tanh,
                                     bias=bt[:, 0:1], scale=1.0)
                nc.sync.dma_start(yv[i], ot[:])
    nc.compile()
    return nc


def _scan_safe(u) -> bool:
    """The ratio recurrence divides by every in-window u[c]; guard the
    amplification of intermediate states."""
    w = np.abs(np.asarray(u, np.float64)[C_LO:C_HI])
    if w.min() <= 0.0:
        return False
    amp = np.cumsum(w) / w
    return float(amp.max()) < 1e5


def build_main(b_core: int, u, c0: float):
    """The nc used for the standard 1M-row problem (also used by test.py
    for tracing)."""
    return _build_split(b_core, u, c0, gps_set=(7, 10), bufs=5)


def kernel(**inputs) -> np.ndarray:
    price = np.ascontiguousarray(np.asarray(inputs["price_series"],
                                            dtype=np.float32))
    B = price.shape[0]
    assert B % N_CORES == 0
    b_core = B // N_CORES

    u, c0 = _collapsed_weights(
        inputs["w_fast"], inputs["b_fast"], inputs["w_slow"],
        inputs["b_slow"], inputs["w_sig"], inputs["b_sig"],
        inputs["norm_scale"], inputs["norm_bias"])

    T = b_core // P if b_core % P == 0 else 0
    if T >= 256 and (T - 128) % 128 == 0:
        nc = build_main(b_core, u, c0)
        in_maps = [{"x": price[i * b_core:(i + 1) * b_core]}
                   for i in range(N_CORES)]
    else:
        nc = _build(b_core, c0, r=max(1, min(64, b_core // P)))
        w_rep = np.ascontiguousarray(
            np.broadcast_to(u[C_LO:C_HI][None, :].astype(np.float32), (P, C)))
        in_maps = [{"x": price[i * b_core:(i + 1) * b_core], "w": w_rep}
                   for i in range(N_CORES)]
    res = run_bass_kernel_spmd(nc, in_maps, list(range(N_CORES)))
    out = np.concatenate([res.results[i]["y"].reshape(-1)
                          for i in range(N_CORES)])
    return out.reshape(B, 1).astype(np.float32)
